# revision 16
# baseline (speedup 1.0000x reference)
import numpy as np

B, N, D = 32, 128, 512
DIR = 2
L = 16
LH = 15
NC = 8
BPC = B // NC
BN = BPC * N
P = 128
KC = D // P
N_WARM = 4

_prog_cache: dict = {}


def _build():
    import concourse.bass as bass
    import concourse.mybir as mybir
    import concourse.tile as tile
    from concourse import bacc

    f32 = mybir.dt.float32
    bf16 = mybir.dt.bfloat16

    nc = bacc.Bacc(
        "TRN2",
        target_bir_lowering=False,
        debug=False,
        num_devices=NC,
    )

    gT_d = nc.dram_tensor("gT", [P, BN], bf16, kind="ExternalInput").ap()
    FT_d = nc.dram_tensor("FT", [P, KC, BN], bf16, kind="ExternalInput").ap()
    WT_d = nc.dram_tensor("WT", [P, KC, D], bf16, kind="ExternalInput").ap()
    esel_d = nc.dram_tensor("esel", [P, LH * LH], bf16, kind="ExternalInput").ap()
    bias_d = nc.dram_tensor("bias2", [L, D], bf16, kind="ExternalInput").ap()
    out = nc.dram_tensor("out", [BPC, N, D], bf16, kind="ExternalOutput").ap()

    with tile.TileContext(nc) as tc:
        with (
            tc.tile_pool(name="work", bufs=1) as wpool,
            tc.tile_pool(name="psum", bufs=1, space="PSUM") as ppool,
        ):
            gT = wpool.tile([P, BN], bf16)
            hb = BN // 2
            nc.sync.dma_start(out=gT[:, 0:hb], in_=gT_d[:, 0:hb])
            nc.scalar.dma_start(out=gT[:, hb:BN], in_=gT_d[:, hb:BN])

            esel = wpool.tile([P, LH * LH], bf16)
            nc.sync.dma_start(out=esel, in_=esel_d)

            WT = wpool.tile([P, KC, D], bf16)
            nc.scalar.dma_start(out=WT, in_=WT_d)

            FT = wpool.tile([P, KC, BN], bf16)
            nc.sync.dma_start(out=FT, in_=FT_d)

            bias_sb = wpool.tile([L, D], bf16)
            nc.scalar.dma_start(out=bias_sb, in_=bias_d)

            warmw = wpool.tile([P, D], bf16)
            nc.gpsimd.memset(warmw[:, 0:P], 0.0)
            nc.vector.memset(warmw[:, P:D], 0.0)
            psum_warm = ppool.tile([P, D], f32, tag="warm", bufs=1)
            for _ in range(N_WARM):
                nc.tensor.matmul(
                    out=psum_warm,
                    lhsT=warmw[:, 0:P],
                    rhs=warmw,
                    start=True,
                    stop=True,
                )

            act_warm = wpool.tile([P, 2], f32)
            nc.scalar.copy(out=act_warm[:, 0:1], in_=warmw[:, 0:1])

            cntT = wpool.tile([L, BN], bf16)
            nc.vector.memset(cntT, 1.0)

            EQ = wpool.tile([P, LH, BN], bf16)
            psum_cnt = ppool.tile([LH, BN], f32, tag="cnt", bufs=1)

            def emit_eq(l):
                nc.vector.tensor_scalar(
                    out=EQ[:, l, :],
                    in0=gT,
                    scalar1=float(l),
                    scalar2=None,
                    op0=mybir.AluOpType.is_equal,
                )

            def emit_cnt(l):
                nc.tensor.matmul(
                    out=psum_cnt,
                    lhsT=esel[:, l * LH : (l + 1) * LH],
                    rhs=EQ[:, l, :],
                    start=(l == 0),
                    stop=(l == LH - 1),
                )

            psum_outs = [
                ppool.tile([P, D], f32, tag="out", bufs=BPC, name=f"psum_out{b}")
                for b in range(BPC)
            ]

            def emit_main(i):
                b, c = divmod(i, KC)
                nc.tensor.matmul(
                    out=psum_outs[b],
                    lhsT=FT[:, c, b * P : (b + 1) * P],
                    rhs=WT[:, c, :],
                    start=(c == 0),
                    stop=False,
                )

            for l in range(LH):
                emit_eq(l)
                emit_cnt(l)

            nc.scalar.copy(out=cntT[0:LH, :], in_=psum_cnt)

            out_sb = wpool.tile([P, BPC, D], bf16)
            h = D // 2
            for b in range(BPC):
                for c in range(KC):
                    emit_main(b * KC + c)
                sl = slice(b * P, (b + 1) * P)
                nc.tensor.matmul(
                    out=psum_outs[b],
                    lhsT=cntT[:, sl],
                    rhs=bias_sb,
                    start=False,
                    stop=True,
                )
                nc.vector.tensor_copy(out=out_sb[:, b, 0:h], in_=psum_outs[b][:, 0:h])
                nc.scalar.copy(out=out_sb[:, b, h:D], in_=psum_outs[b][:, h:D])
                ring = nc.sync if b % 2 == 0 else nc.scalar
                if b == BPC - 1:
                    nc.sync.dma_start(out=out[b, :, 0:h], in_=out_sb[:, b, 0:h])
                    nc.scalar.dma_start(out=out[b, :, h:D], in_=out_sb[:, b, h:D])
                else:
                    ring.dma_start(out=out[b], in_=out_sb[:, b, :])

    nc.compile()
    return nc


def _get_prog():
    if "p" not in _prog_cache:
        _prog_cache["p"] = _build()
    return _prog_cache["p"]


def _shard_inputs(feature, graph, weights, bias):
    import ml_dtypes

    bf16 = ml_dtypes.bfloat16

    feature = np.asarray(feature, dtype=np.float32)
    weights = np.asarray(weights, dtype=np.float32)
    bias = np.asarray(bias, dtype=np.float32)
    g = np.asarray(graph)
    if g.dtype == np.int64:
        g32 = g.view(np.int32)[..., ::2]
    else:
        g32 = g.astype(np.int32)

    M = weights.sum(axis=0) + np.eye(D, dtype=np.float32)
    WT_h = np.ascontiguousarray(M.T.reshape(KC, P, D).transpose(1, 0, 2).astype(bf16))

    esel_h = np.zeros((P, LH, LH), dtype=bf16)
    idx = np.arange(LH)
    esel_h[:, idx, idx] = 1.0
    esel_h = esel_h.reshape(P, LH * LH)

    bias2 = bias - bias[L - 1]
    bias2[L - 1] = N * bias[L - 1]
    bias_h = bias2.astype(bf16)

    in_maps = []
    for core in range(NC):
        sl = slice(core * BPC, (core + 1) * BPC)
        Fc = feature[sl].reshape(BN, D)
        FT_h = np.ascontiguousarray(Fc.T.reshape(KC, P, BN).transpose(1, 0, 2).astype(bf16))
        gc = g32[sl].reshape(BN, N)
        gT_h = np.ascontiguousarray(gc.T.astype(bf16))
        in_maps.append(
            {"gT": gT_h, "FT": FT_h, "WT": WT_h, "esel": esel_h, "bias2": bias_h}
        )
    return in_maps


def _run(feature, graph, weights, bias, trace=False):
    from concourse.bass_utils import run_bass_kernel_spmd

    in_maps = _shard_inputs(feature, graph, weights, bias)
    nc = _get_prog()
    res = run_bass_kernel_spmd(nc, in_maps, core_ids=list(range(NC)), trace=trace)
    out = np.concatenate(
        [np.asarray(r["out"]).astype(np.float32) for r in res.results], axis=0
    )
    return out, res


def kernel(feature, graph, weights, bias):
    out, _ = _run(feature, graph, weights, bias, trace=False)
    return out


# revision 19
# speedup vs baseline: 1.0963x; 1.0963x over previous
import numpy as np

B, N, D = 32, 128, 512
DIR = 2
L = 16
LH = 15
NC = 8
BPC = B // NC
BN = BPC * N
P = 128
KC = D // P
N_WARM = 4

_prog_cache: dict = {}


def _build():
    import concourse.bass as bass
    import concourse.mybir as mybir
    import concourse.tile as tile
    from concourse import bacc

    f32 = mybir.dt.float32
    bf16 = mybir.dt.bfloat16
    fp8 = mybir.dt.float8e4

    nc = bacc.Bacc(
        "TRN2",
        target_bir_lowering=False,
        debug=False,
        num_devices=NC,
    )

    gT_d = nc.dram_tensor("gT", [P, BN], bf16, kind="ExternalInput").ap()
    FT_d = nc.dram_tensor("FT", [P, KC, BN], bf16, kind="ExternalInput").ap()
    WT_d = nc.dram_tensor("WT", [P, KC, D], bf16, kind="ExternalInput").ap()
    esel_d = nc.dram_tensor("esel", [P, (LH // 2 + 1) * 2 * 16], fp8, kind="ExternalInput").ap()
    bias_d = nc.dram_tensor("bias2", [L, D], bf16, kind="ExternalInput").ap()
    out = nc.dram_tensor("out", [BPC, N, D], bf16, kind="ExternalOutput").ap()

    with tile.TileContext(nc) as tc:
        with (
            tc.tile_pool(name="work", bufs=1) as wpool,
            tc.tile_pool(name="psum", bufs=1, space="PSUM") as ppool,
        ):
            gT = wpool.tile([P, BN], bf16)
            hb = BN // 2
            nc.sync.dma_start(out=gT[:, 0:hb], in_=gT_d[:, 0:hb])
            nc.scalar.dma_start(out=gT[:, hb:BN], in_=gT_d[:, hb:BN])

            esel = wpool.tile([P, LH // 2 + 1, 2, 16], fp8)
            nc.sync.dma_start(
                out=esel.rearrange("p g k j -> p (g k j)"), in_=esel_d
            )

            WT = wpool.tile([P, KC, D], bf16)
            nc.scalar.dma_start(out=WT, in_=WT_d)

            FT = wpool.tile([P, KC, BN], bf16)
            nc.sync.dma_start(out=FT, in_=FT_d)

            bias_sb = wpool.tile([L, D], bf16)
            nc.scalar.dma_start(out=bias_sb, in_=bias_d)

            warmw = wpool.tile([P, D], bf16)
            nc.gpsimd.memset(warmw[:, 0:P], 0.0)
            nc.vector.memset(warmw[:, P:D], 0.0)
            psum_warm = ppool.tile([P, D], f32, tag="warm", bufs=1)
            for _ in range(8):
                nc.tensor.matmul(
                    out=psum_warm[:, 0:P],
                    lhsT=warmw[:, 0:P],
                    rhs=warmw[:, 0:P],
                    start=True,
                    stop=True,
                )
            for _ in range(6):
                nc.tensor.matmul(
                    out=psum_warm,
                    lhsT=warmw[:, 0:P],
                    rhs=warmw,
                    start=True,
                    stop=True,
                )

            act_warm = wpool.tile([P, 2], f32)
            nc.scalar.copy(out=act_warm[:, 0:1], in_=warmw[:, 0:1])

            cntT = wpool.tile([L, BN], bf16)
            nc.vector.memset(cntT, 1.0)

            EQ = wpool.tile([P, LH, BN], fp8)
            psum_cnt = ppool.tile([L, BN], f32, tag="cnt", bufs=1)
            NPAIR = LH // 2

            def emit_eq(l):
                nc.vector.tensor_scalar(
                    out=EQ[:, l, :],
                    in0=gT,
                    scalar1=float(l),
                    scalar2=None,
                    op0=mybir.AluOpType.is_equal,
                )

            def emit_cnt_pair(g):
                nc.tensor.matmul(
                    out=psum_cnt,
                    lhsT=esel[:, g, :, :],
                    rhs=EQ[:, 2 * g : 2 * g + 2, :],
                    start=(g == 0),
                    stop=False,
                    perf_mode=mybir.MatmulPerfMode.DoubleRow,
                )

            psum_outs = [
                ppool.tile([P, D], f32, tag="out", bufs=BPC, name=f"psum_out{b}")
                for b in range(BPC)
            ]

            def emit_main(i):
                b, c = divmod(i, KC)
                nc.tensor.matmul(
                    out=psum_outs[b],
                    lhsT=FT[:, c, b * P : (b + 1) * P],
                    rhs=WT[:, c, :],
                    start=(c == 0),
                    stop=False,
                )

            mi = 0
            for g in range(NPAIR):
                emit_eq(2 * g)
                emit_eq(2 * g + 1)
                emit_cnt_pair(g)
                if g > 0:
                    emit_main(mi)
                    mi += 1
            emit_eq(LH - 1)
            nc.tensor.matmul(
                out=psum_cnt,
                lhsT=esel[:, NPAIR, 0, :],
                rhs=EQ[:, LH - 1, :],
                start=False,
                stop=True,
            )
            nc.scalar.copy(out=cntT[0:LH, :], in_=psum_cnt[0:LH, :])


            out_sb = wpool.tile([P, BPC, D], bf16)
            h = D // 2

            def emit_tail(b):
                sl = slice(b * P, (b + 1) * P)
                nc.tensor.matmul(
                    out=psum_outs[b],
                    lhsT=cntT[:, sl],
                    rhs=bias_sb,
                    start=False,
                    stop=True,
                )
                nc.vector.tensor_copy(out=out_sb[:, b, 0:h], in_=psum_outs[b][:, 0:h])
                nc.scalar.copy(out=out_sb[:, b, h:D], in_=psum_outs[b][:, h:D])
                if b == BPC - 1:
                    nc.sync.dma_start(out=out[b, :, 0:h], in_=out_sb[:, b, 0:h])
                    nc.scalar.dma_start(out=out[b, :, h:D], in_=out_sb[:, b, h:D])
                else:
                    ring = nc.sync if b % 2 == 0 else nc.scalar
                    ring.dma_start(out=out[b], in_=out_sb[:, b, :])

            emit_tail(0)
            while mi < BPC * KC:
                emit_main(mi)
                mi += 1
                if mi % KC == 0:
                    emit_tail(mi // KC - 1)

    nc.compile()
    return nc


def _get_prog():
    if "p" not in _prog_cache:
        _prog_cache["p"] = _build()
    return _prog_cache["p"]


def _shard_inputs(feature, graph, weights, bias):
    import ml_dtypes

    bf16 = ml_dtypes.bfloat16

    feature = np.asarray(feature, dtype=np.float32)
    weights = np.asarray(weights, dtype=np.float32)
    bias = np.asarray(bias, dtype=np.float32)
    g = np.asarray(graph)
    if g.dtype == np.int64:
        g32 = g.view(np.int32)[..., ::2]
    else:
        g32 = g.astype(np.int32)

    M = weights.sum(axis=0) + np.eye(D, dtype=np.float32)
    WT_h = np.ascontiguousarray(M.T.reshape(KC, P, D).transpose(1, 0, 2).astype(bf16))

    import concourse.mybir as mybir

    fp8np = mybir.dt.np(mybir.dt.float8e4)
    NG = LH // 2 + 1
    esel_h = np.zeros((P, NG, 2, 16), dtype=np.float32)
    for lab in range(LH):
        esel_h[:, lab // 2, lab % 2, lab] = 1.0
    esel_h = esel_h.reshape(P, NG * 2 * 16).astype(fp8np)

    bias2 = bias - bias[L - 1]
    bias2[L - 1] = N * bias[L - 1]
    bias_h = bias2.astype(bf16)

    in_maps = []
    for core in range(NC):
        sl = slice(core * BPC, (core + 1) * BPC)
        Fc = feature[sl].reshape(BN, D)
        FT_h = np.ascontiguousarray(Fc.T.reshape(KC, P, BN).transpose(1, 0, 2).astype(bf16))
        gc = g32[sl].reshape(BN, N)
        gT_h = np.ascontiguousarray(gc.T.astype(bf16))
        in_maps.append(
            {"gT": gT_h, "FT": FT_h, "WT": WT_h, "esel": esel_h, "bias2": bias_h}
        )
    return in_maps


def _run(feature, graph, weights, bias, trace=False):
    from concourse.bass_utils import run_bass_kernel_spmd

    in_maps = _shard_inputs(feature, graph, weights, bias)
    nc = _get_prog()
    res = run_bass_kernel_spmd(nc, in_maps, core_ids=list(range(NC)), trace=trace)
    out = np.concatenate(
        [np.asarray(r["out"]).astype(np.float32) for r in res.results], axis=0
    )
    return out, res


def kernel(feature, graph, weights, bias):
    out, _ = _run(feature, graph, weights, bias, trace=False)
    return out
